# revision 22
# baseline (speedup 1.0000x reference)
"""DGCN forward kernel for Trainium2, 8-core data-parallel over batch.

Reference computation (per batch b):
    x1_s = a_s[b]^T @ X          (X = x[b] viewed [N=512, T*C=768])
    x2_s = a_s[b]^T @ x1_s
    h = concat([X, x1_0, x2_0, x1_1, x2_1, x1_2, x2_2], channel)   # [N,T,448]
    y = h @ W^T + bias

We commute the (linear) 1x1-conv projection past the diffusions:
    Z_k = X @ W_k^T   (W_k = W[:, 64k:64(k+1)]), k = 0..6
    y[b] = Z0 + A0^T (Z1 + A0^T Z2) + A1^T (Z3 + A1^T Z4) + A2^T (Z5 + A2^T Z6) + b

so the projection touches only X (K=64 matmuls against a repacked V, row-tiled
in pairs over the PE array), and every elementwise add fuses into the
mandatory PSUM->SBUF copies. x is pre-transposed on the host so the kernel
needs no on-device transposes. The bias lands in the final-PSUM via a K=1
ones-row matmul.

V's 7 channel blocks are reordered [2,4,6 | 1,3,5 | 0] so that the
"even" blocks (diffusion-matmul inputs, short lifetime -> zsE) and the
"odd + 0" blocks (add-inputs, long lifetime -> zsL) are each contiguous
in the projection PSUM output.

Emission is software-pipelined: loads+projection of batch i+1 are emitted
before the diffusion stages of batch i, so the PE never waits on the
PSUM->SBUF copy engines (DVE/ACT) across batch boundaries.

mm_dtype="f32r" uses TF32-mode matmuls (4x faster than fp32 on TRN2);
mm_dtype="f32" uses full-precision fp32 matmuls.

Shapes (hardcoded): B=32, N=512, T=12, C=64, c_out=64, 8 cores x 4 batches.
"""

import numpy as np

import concourse.bass as bass  # noqa: F401
import concourse.mybir as mybir
import concourse.tile as tile
from concourse import bacc
from concourse.bass_utils import run_bass_kernel_spmd

F32 = mybir.dt.float32
F32R = mybir.dt.float32r

B, N, T, C = 32, 512, 12, 64
NCORES = 8
BPC = B // NCORES          # batches per core
NT = N // 128              # node tiles
TC = T * C                 # 768
HALF = TC // 2             # 384  (PSUM bank-sized slice of the (t,o) free dim)
NK = 7                     # projection channel blocks
# V block order in the projection output: evens (diffusion rhs), odds, k0
KORDER = [2, 4, 6, 1, 3, 5, 0]

DEFAULT_MM_DTYPE = "f32r"
DEFAULT_HOST_ROUND = True


def build_program(rep=1, mm_dtype=DEFAULT_MM_DTYPE, loop_iters=None):
    """Build + compile the per-core Bass program. rep>1 repeats the whole
    4-batch pipeline (python-unrolled); loop_iters wraps the body in an
    on-device For_i loop (for timing amortization)."""
    mm_dt = F32R if mm_dtype == "f32r" else F32

    def asf32(ap):
        return ap.bitcast(F32) if mm_dt != F32 else ap

    nc = bacc.Bacc("TRN2", target_bir_lowering=False, debug=False)

    xt_d = nc.dram_tensor("xt", [BPC, 128, 6 * N], mm_dt, kind="ExternalInput")
    a_d = [
        nc.dram_tensor(f"a{s}", [BPC, NT, 128, N], mm_dt, kind="ExternalInput")
        for s in range(3)
    ]
    v_d = nc.dram_tensor("v2", [128, NK * 64], mm_dt, kind="ExternalInput")
    ones_d = nc.dram_tensor("ones1", [1, 128], mm_dt, kind="ExternalInput")
    biasrow_d = nc.dram_tensor("biasrow", [1, TC], mm_dt, kind="ExternalInput")
    y_d = nc.dram_tensor("y", [BPC, N, TC], F32, kind="ExternalOutput")

    with tile.TileContext(nc) as tc:
        with (
            tc.tile_pool(name="consts", bufs=1) as cpool,
            tc.tile_pool(name="xt", bufs=2) as xt_pool,
            tc.tile_pool(name="a", bufs=16) as a_pool,
            tc.tile_pool(name="zse", bufs=4) as zse_pool,
            tc.tile_pool(name="zsl", bufs=5) as zsl_pool,
            tc.tile_pool(name="u", bufs=12) as u_pool,
            tc.tile_pool(name="y", bufs=3) as y_pool,
            tc.tile_pool(name="psz", bufs=2, space="PSUM") as psz_pool,
            tc.tile_pool(name="psa", bufs=2, space="PSUM") as psa_pool,
        ):
            v_sb = cpool.tile([128, NK * 64], mm_dt, tag="v2")
            nc.sync.dma_start(v_sb[:], v_d.ap()[:])
            ones_sb = cpool.tile([1, 128], mm_dt, tag="ones")
            nc.sync.dma_start(ones_sb[:], ones_d.ap()[:])
            biasrow_sb = cpool.tile([1, TC], mm_dt, tag="biasrow")
            nc.sync.dma_start(biasrow_sb[:], biasrow_d.ap()[:])

            # per-batch-index live tile state (keyed by pipeline index)
            state = {}

            def emit_loads(i, b):
                xt_sb = xt_pool.tile([128, 6 * N], mm_dt, tag="xt",
                                     name=f"xt_{i}")
                nc.sync.dma_start(xt_sb[:], xt_d.ap()[b])
                a_sb = {}
                for s in range(3):
                    for vt in range(NT):
                        at = a_pool.tile([128, N], mm_dt, tag="a",
                                         name=f"a_{i}_{s}_{vt}")
                        nc.sync.dma_start(at[:], a_d[s].ap()[b, vt])
                        a_sb[s, vt] = at
                state[i] = dict(xt=xt_sb, a=a_sb, b=b)

            def front_units(i):
                """Projection work units (one t-pair each): 2 MMs + 2 copies.

                zsE[nt] free layout: (ke, t, o); ke = 0,1,2 ~ k = 2,4,6
                zsL[nt] free layout: (kl, t, o); kl = 0,1,2 ~ k = 1,3,5; kl=3 ~ k=0
                """
                st = state[i]
                xt_sb = st["xt"]
                zse, zsl = [], []
                st["zse"], st["zsl"] = zse, zsl

                def unit(nt, j):
                    if j == 0:
                        # lazily allocate this nt's Z tiles at first use
                        zse.append(zse_pool.tile([128, 3 * TC], mm_dt, tag="zse",
                                                 name=f"zse_{i}_{nt}"))
                        zsl.append(zsl_pool.tile([128, 4 * TC], mm_dt, tag="zsl",
                                                 name=f"zsl_{i}_{nt}"))
                    ze3 = zse[nt][:].rearrange("p (k f) -> p k f", k=3)
                    zl3 = zsl[nt][:].rearrange("p (k f) -> p k f", k=4)
                    zp = psz_pool.tile([128, 1024], F32, tag="psz",
                                       name=f"zp_{i}_{nt}_{j}")
                    col = j * N + nt * 128
                    for h in range(2):
                        nc.tensor.matmul(
                            zp[:, h * 512:h * 512 + NK * 64],
                            lhsT=xt_sb[h * 64:(h + 1) * 64, col:col + 128],
                            rhs=v_sb[h * 64:(h + 1) * 64, :],
                            start=True, stop=True,
                        )
                    zp4 = zp[:].rearrange("p (h f) -> p h f", h=2)
                    zpE = zp4[:, :, 0:192].rearrange("p h (k o) -> p h k o", k=3)
                    zpL = zp4[:, :, 192:448].rearrange("p h (k o) -> p h k o", k=4)
                    t0 = 2 * j
                    dstE = ze3[:, :, t0 * 64:(t0 + 2) * 64].rearrange(
                        "p k (h o) -> p h k o", h=2)
                    dstL = zl3[:, :, t0 * 64:(t0 + 2) * 64].rearrange(
                        "p k (h o) -> p h k o", h=2)
                    nc.vector.tensor_copy(out=dstE, in_=zpE)
                    nc.scalar.copy(dstL, zpL)

                return [(lambda nt=nt, j=j: unit(nt, j))
                        for nt in range(NT) for j in range(6)]

            def emit_u(i):
                """First diffusion: U'_s = A_s^T Z_{2s+2} + Z_{2s+1}."""
                st = state[i]
                a_sb, zse, zsl = st["a"], st["zse"], st["zsl"]
                u_sb = {}
                st["u"] = u_sb
                for s in range(3):
                    for wt in range(NT):
                        ut = u_pool.tile([128, TC], mm_dt, tag="u",
                                         name=f"u_{i}_{s}_{wt}")
                        u_sb[s, wt] = ut
                        up = psa_pool.tile([128, 1024], F32, tag="psa",
                                           name=f"up_{i}_{s}_{wt}")
                        for h in range(2):
                            off_e = s * TC + h * HALF
                            for kt in range(NT):
                                nc.tensor.matmul(
                                    up[:, h * 512:h * 512 + HALF],
                                    lhsT=a_sb[s, kt][:, wt * 128:(wt + 1) * 128],
                                    rhs=zse[kt][:, off_e:off_e + HALF],
                                    start=(kt == 0), stop=(kt == NT - 1),
                                )
                        off_o = s * TC
                        nc.vector.tensor_tensor(
                            ut[:].rearrange("p (h f) -> p h f", h=2),
                            up[:].rearrange("p (h f) -> p h f", h=2)[:, :, :HALF],
                            asf32(zsl[wt][:, off_o:off_o + TC]).rearrange(
                                "p (h f) -> p h f", h=2),
                            mybir.AluOpType.add,
                        )

            def y_units(i):
                """Second diffusion + combine, as interleavable chunks.

                Per wt: 4 chunks of MMs (bias + 26 MMs split across the two
                halves); last chunk appends the combine add + output DMA.
                """
                st = state[i]
                a_sb, zsl, u_sb, b = st["a"], st["zsl"], st["u"], st["b"]
                units = []
                for wt in range(NT):
                    tiles = {}

                    def alloc(wt=wt, tiles=tiles):
                        tiles["yt"] = y_pool.tile([128, TC], F32, tag="y",
                                                  name=f"y_{i}_{wt}")
                        tiles["yp"] = psa_pool.tile([128, 1024], F32, tag="psa",
                                                    name=f"yp_{i}_{wt}")

                    # flat MM list for this wt: (h, s, kt) with bias first
                    mms = []
                    for h in range(2):
                        mms.append(("bias", h))
                        for s in range(3):
                            for kt in range(NT):
                                mms.append((h, s, kt))

                    def run_chunk(chunk, tiles=tiles, wt=wt):
                        yp = tiles["yp"]
                        for item in chunk:
                            if item[0] == "bias":
                                h = item[1]
                                nc.tensor.matmul(
                                    yp[:, h * 512:h * 512 + HALF],
                                    lhsT=ones_sb[:],
                                    rhs=biasrow_sb[:, h * HALF:(h + 1) * HALF],
                                    start=True, stop=False,
                                )
                            else:
                                h, s, kt = item
                                nc.tensor.matmul(
                                    yp[:, h * 512:h * 512 + HALF],
                                    lhsT=a_sb[s, kt][:, wt * 128:(wt + 1) * 128],
                                    rhs=u_sb[s, kt][:, h * HALF:(h + 1) * HALF],
                                    start=False,
                                    stop=(s == 2 and kt == NT - 1),
                                )

                    def finish(tiles=tiles, wt=wt, b=b):
                        yt, yp = tiles["yt"], tiles["yp"]
                        nc.vector.tensor_tensor(
                            yt[:].rearrange("p (h f) -> p h f", h=2),
                            yp[:].rearrange("p (h f) -> p h f", h=2)[:, :, :HALF],
                            asf32(zsl[wt][:, 3 * TC:4 * TC]).rearrange(
                                "p (h f) -> p h f", h=2),
                            mybir.AluOpType.add,
                        )
                        nc.sync.dma_start(
                            y_d.ap()[b, wt * 128:(wt + 1) * 128, :], yt[:]
                        )

                    CH = 7  # MMs per chunk
                    chunks = [mms[q:q + CH] for q in range(0, len(mms), CH)]
                    for ci, ch in enumerate(chunks):
                        first = (ci == 0)
                        last = (ci == len(chunks) - 1)
                        units.append(
                            (lambda ch=ch, first=first, last=last, alloc=alloc,
                                    run_chunk=run_chunk, finish=finish:
                             (alloc() if first else None,
                              run_chunk(ch),
                              finish() if last else None)))
                return units

            def interleave(ua, ub):
                """Proportionally merge two unit lists (ub slightly leading)."""
                out = []
                na, nb = len(ua), len(ub)
                ia = ib = 0
                while ia < na or ib < nb:
                    # emit from ub when its progress fraction lags
                    if ib < nb and (ia >= na or ib * na <= ia * nb):
                        out.append(ub[ib]); ib += 1
                    else:
                        out.append(ua[ia]); ia += 1
                return out

            def emit_pipeline():
                n = rep * BPC
                emit_loads(0, 0)
                fu = front_units(0)
                for f in fu:
                    f()
                for i in range(n):
                    if i + 1 < n:
                        emit_loads(i + 1, (i + 1) % BPC)
                    emit_u(i)
                    yu = y_units(i)
                    fu = front_units(i + 1) if i + 1 < n else []
                    for f in interleave(fu, yu):
                        f()
                    state.pop(i)

            if loop_iters is None:
                emit_pipeline()
            else:
                with tc.For_i(0, loop_iters, 1,
                              hint_engines=(mybir.EngineType.PE,
                                            mybir.EngineType.DVE,
                                            mybir.EngineType.SP,
                                            mybir.EngineType.Activation,
                                            mybir.EngineType.Pool)):
                    emit_pipeline()

    nc.compile()
    return nc


def tf32_round(arr):
    """Round fp32 to TF32 (10-bit mantissa), round-to-nearest-even."""
    u = np.ascontiguousarray(arr).view(np.uint32)
    lsb = (u >> np.uint32(13)) & np.uint32(1)
    r = u + np.uint32(0x0FFF) + lsb
    return (r & np.uint32(0xFFFFE000)).view(np.float32)


def prep_inputs(x, a0, a1, a2, W, b, mm_dtype=DEFAULT_MM_DTYPE,
                host_round=DEFAULT_HOST_ROUND):
    """Host-side shard + repack. Returns per-core in_maps."""
    x = np.ascontiguousarray(np.asarray(x, dtype=np.float32))
    aa = [np.ascontiguousarray(np.asarray(a, dtype=np.float32)) for a in (a0, a1, a2)]
    W = np.asarray(W, dtype=np.float32)
    b = np.asarray(b, dtype=np.float32)

    rnd = tf32_round if (mm_dtype == "f32r" and host_round) else (lambda v: v)

    # V[c, q*64+o] = W[o, KORDER[q]*64+c]; duplicated across both halves.
    Vk = W.reshape(64, NK, 64).transpose(2, 1, 0)        # [c, k, o]
    V = Vk[:, KORDER, :].reshape(64, NK * 64)
    v2 = rnd(np.ascontiguousarray(np.concatenate([V, V], axis=0)))
    ones1 = np.ones((1, 128), dtype=np.float32)
    biasrow = rnd(np.ascontiguousarray(np.tile(b, T)[None, :]))

    in_maps = []
    for ci in range(NCORES):
        sl = slice(ci * BPC, (ci + 1) * BPC)
        xs = x[sl]  # [BPC, N, T, C]
        # xt[b, h*64+c, j*512+n] = x[b, n, 2j+h, c]
        xt = np.ascontiguousarray(
            xs.reshape(BPC, N, 6, 2, C).transpose(0, 3, 4, 2, 1)
        ).reshape(BPC, 128, 6 * N)
        m = {
            "xt": rnd(xt),
            "v2": v2,
            "ones1": ones1,
            "biasrow": biasrow,
        }
        for s in range(3):
            m[f"a{s}"] = rnd(np.ascontiguousarray(aa[s][sl].reshape(BPC, NT, 128, N)))
        in_maps.append(m)
    return in_maps


def gather_output(results):
    """results: list of per-core {'y': [BPC, N, TC]} -> [B, N, T, C]."""
    ys = [results[ci]["y"].reshape(BPC, N, T, C) for ci in range(NCORES)]
    return np.ascontiguousarray(np.concatenate(ys, axis=0))


_PROGRAM_CACHE = {}


def kernel(x, a0, a1, a2, W, b):
    key = (1, DEFAULT_MM_DTYPE)
    if key not in _PROGRAM_CACHE:
        _PROGRAM_CACHE[key] = build_program(rep=key[0], mm_dtype=key[1])
    nc = _PROGRAM_CACHE[key]
    in_maps = prep_inputs(x, a0, a1, a2, W, b, mm_dtype=DEFAULT_MM_DTYPE)
    res = run_bass_kernel_spmd(nc, in_maps, core_ids=list(range(NCORES)))
    return gather_output(res.results)


if __name__ == "__main__":
    rng = np.random.default_rng(0)
    x = rng.standard_normal((B, N, T, C), dtype=np.float32)
    a0 = rng.random((B, N, N), dtype=np.float32)
    a1 = rng.random((B, N, N), dtype=np.float32)
    a2 = rng.random((B, N, N), dtype=np.float32)
    W = (rng.standard_normal((64, 448), dtype=np.float32) * 0.05).astype(np.float32)
    b = (rng.standard_normal((64,), dtype=np.float32) * 0.05).astype(np.float32)
    y = kernel(x, a0, a1, a2, W, b)
    print("y shape", y.shape, "mean", y.mean())
